# revision 13
# baseline (speedup 1.0000x reference)
"""CapsuleLayer dynamic-routing kernel for 8 Trainium2 NeuronCores. v3

Problem: x[32, 2048, 16], W[1, 2048, 64, 32, 16] -> v[32, 64, 32]
  u_hat = einsum('iodk,bik->biod', W[0], x)
  3 routing iterations (softmax over out_caps, squash over out_dim).

Sharding: in_caps split 8 ways (256/core); W resident in SBUF bf16; s_j
AllReduced per routing pass (only cross-core quantity).

v3 design:
 - columns in (d, o) order, o innermost: the softmax scale e''[p,o]
   broadcasts over the outer d dim keeping step-1 inner -> every wide DVE
   op is a 2x-mode bf16 tensor_tensor (measured ~1.14us per [128,2048]).
 - agreement d-reduction runs on the PE: 32 accumulating identity-matmuls
   (rhs = tmp[:, d, :], lhsT = I) sum the d-slices into an f32 PSUM
   [128,64] tile at 29ns/MM (LDWEIGHTS of the repeated identity pipelines
   through the background weight buffer). Replaces a 2.1us DVE tree.
 - s accumulates in ONE psum bank as [(ch,b), 512] via col-offset
   tile_position selector matmuls (4 col-groups run concurrently, ~330ns).
 - software-pipelined emission: evac(q+1) is queued on ACT before exp(q),
   and mul(q+1) is queued on DVE before recip(q), so neither engine idles
   during the cross-engine ping-pong.
 - no GpSimd elementwise (shares SBUF port with DVE).
 - squash uses sqrt(n2) = exp(0.5*ln(n2)): stays on one ACT table set.
"""

import numpy as np
import ml_dtypes

B, IC, KD, OC, OD = 32, 2048, 16, 64, 32
NCORES = 8
ICC = IC // NCORES                            # 256 in_caps per core
NJ = ICC // 8                                 # 32 tau blocks (8 i per block)
OD2 = OC * OD                                 # 2048 flattened cols, (d, o) order
NQ = 2 * NJ                                   # 64 quads (4 i each)
NUM_ROUTES = 3

_CACHE = {}


def _build_program():
    import concourse.bacc as bacc
    import concourse.tile as tile
    import concourse.mybir as mybir

    f32 = mybir.dt.float32
    bf16 = mybir.dt.bfloat16
    ALU = mybir.AluOpType
    ACTF = mybir.ActivationFunctionType

    nc = bacc.Bacc("TRN2", target_bir_lowering=False, debug=False, num_devices=NCORES)

    WL_d = nc.dram_tensor("WL", [128, NJ * OD2], bf16, kind="ExternalInput").ap()
    xS0_d = nc.dram_tensor("xS0", [128, NJ * B], bf16, kind="ExternalInput").ap()
    xS1_d = nc.dram_tensor("xS1", [128, NJ * B], bf16, kind="ExternalInput").ap()
    SEL1_d = nc.dram_tensor("SEL1", [128, 32], bf16, kind="ExternalInput").ap()
    IDN_d = nc.dram_tensor("IDN", [128, 128], bf16, kind="ExternalInput").ap()
    X2_d = nc.dram_tensor("X2", [128, NJ * B], bf16, kind="ExternalInput").ap()
    vout_d = nc.dram_tensor("v_out", [B, OD2], f32, kind="ExternalOutput").ap()

    with tile.TileContext(nc) as tc:
        with (
            tc.tile_pool(name="const", bufs=1) as cp,
            tc.tile_pool(name="work", bufs=2) as wp,
            tc.tile_pool(name="small", bufs=2) as sp,
            tc.tile_pool(name="bound", bufs=1) as bp,
            tc.tile_pool(name="psum", bufs=2, space="PSUM") as pp,
            tc.tile_pool(name="pagr", bufs=2, space="PSUM") as pg,
            tc.tile_pool(name="psacc", bufs=1, space="PSUM") as pa,
            tc.tile_pool(name="dram", bufs=1, space="DRAM") as dp,
        ):
            # ---- resident inputs ----
            wl = cp.tile([128, NJ * OD2], bf16, tag="wl")
            for blk in range(8):
                w = NJ * OD2 // 8
                nc.sync.dma_start(out=wl[:, blk * w:(blk + 1) * w],
                                  in_=WL_d[:, blk * w:(blk + 1) * w])
            xs = [cp.tile([128, NJ * B], bf16, tag=f"xs{s}", name=f"xs{s}") for s in range(2)]
            nc.sync.dma_start(out=xs[0][:, :], in_=xS0_d[:, :])
            nc.sync.dma_start(out=xs[1][:, :], in_=xS1_d[:, :])
            sel1 = cp.tile([128, 32], bf16, tag="sel1")
            nc.sync.dma_start(out=sel1[:, :], in_=SEL1_d[:, :])
            idn = cp.tile([128, 128], bf16, tag="idn")
            nc.sync.dma_start(out=idn[:, :], in_=IDN_d[:, :])
            x2t = cp.tile([128, NJ * B], bf16, tag="x2t")
            nc.sync.dma_start(out=x2t[:, :], in_=X2_d[:, :])

            # ---- persistent state ----
            V4 = cp.tile([128, OD2], bf16, tag="V4")    # Vacc replicated x4 part-groups
            Vacc = cp.tile([B, OD2], bf16, tag="Vacc")  # running sum of v_t, (d,o) cols

            ar_in = [dp.tile([128, 512], f32, tag=f"ari{t}", name=f"ari{t}") for t in range(NUM_ROUTES)]
            ar_out = [dp.tile([128, 512], f32, tag=f"aro{t}", name=f"aro{t}") for t in range(NUM_ROUTES)]

            def emit_quad(t, q):
                """PE u_hat quad q + ACT evacuation -> uhsb (bf16, (d,o))."""
                jj, s_ = divmod(q, 2)
                uhp = [pp.tile([128, 1024], f32, tag="uhp", name=f"uhp{t}_{q}_{h}")
                       for h in range(2)]
                for h in range(2):
                    for ch in range(2):
                        col = jj * OD2 + (2 * h + ch) * 512
                        for r in range(4):
                            nc.tensor.matmul(
                                uhp[h][32 * r:32 * r + 32, ch * 512:(ch + 1) * 512],
                                lhsT=xs[s_][32 * r:32 * r + 32, jj * B:(jj + 1) * B],
                                rhs=wl[32 * r:32 * r + 32, col: col + 512],
                                start=True, stop=True,
                                tile_position=(32 * r, 32 * r),
                            )
                uhsb = wp.tile([128, OD2], bf16, tag="uhb", name=f"uhsb{t}_{q}")
                for h in range(2):
                    nc.scalar.copy(uhsb[:, h * 1024:(h + 1) * 1024], uhp[h][:, :])
                return uhsb

            def emit_mul(t, q, uhsb):
                """DVE tmp = uhsb * V4 (bf16 2x)."""
                tmp = wp.tile([128, OD2], bf16, tag="tmp", name=f"tmp{t}_{q}")
                nc.vector.tensor_tensor(out=tmp[:, :], in0=uhsb[:, :], in1=V4[:, :],
                                        op=ALU.mult)
                return tmp

            def allreduce_s(t, src_psum):
                """Evacuate packed s (psum [128,512] f32) -> allreduce."""
                s_sb = cp.tile([128, 512], f32, tag="ssb", name=f"s_sb{t}")
                nc.scalar.copy(s_sb[:, :], src_psum[:, :])
                nc.sync.dma_start(out=ar_in[t][:, :], in_=s_sb[:, :])
                nc.gpsimd.collective_compute(
                    "AllReduce", ALU.add,
                    replica_groups=[list(range(NCORES))],
                    ins=[ar_in[t].opt()],
                    outs=[ar_out[t].opt()],
                )
                nc.sync.dma_start(out=s_sb[:, :], in_=ar_out[t][:, :])
                # unpack [(ch,b), 512] -> [32, 2048]
                spk = bp.tile([B, OD2], f32, tag="spk", name=f"spk{t}")
                for ch in range(4):
                    nc.sync.dma_start(out=spk[:, ch * 512:(ch + 1) * 512],
                                      in_=s_sb[32 * ch:32 * ch + 32, :])
                return spk

            def squash(t, s_sb):
                """v_t = squash(s_sb [32,2048] f32, (d,o) cols)."""
                sq = bp.tile([B, OD2], bf16, tag="sqv", name=f"sq{t}")
                nc.scalar.activation(sq[:, :], s_sb[:, :], ACTF.Square)
                sqv = sq[:, :].rearrange("p (d o) -> p d o", o=OC)
                q1 = bp.tile([B, 16 * OC], bf16, tag="q1", name=f"q1_{t}")
                nc.vector.tensor_tensor(out=q1[:, :].rearrange("p (d o) -> p d o", o=OC),
                                        in0=sqv[:, 0:16, :], in1=sqv[:, 16:32, :], op=ALU.add)
                q1v = q1[:, :].rearrange("p (d o) -> p d o", o=OC)
                q2 = bp.tile([B, 8 * OC], bf16, tag="q2", name=f"q2_{t}")
                nc.vector.tensor_tensor(out=q2[:, :].rearrange("p (d o) -> p d o", o=OC),
                                        in0=q1v[:, 0:8, :], in1=q1v[:, 8:16, :], op=ALU.add)
                q2v = q2[:, :].rearrange("p (d o) -> p d o", o=OC)
                q3 = bp.tile([B, 4 * OC], bf16, tag="q3", name=f"q3_{t}")
                nc.vector.tensor_tensor(out=q3[:, :].rearrange("p (d o) -> p d o", o=OC),
                                        in0=q2v[:, 0:4, :], in1=q2v[:, 4:8, :], op=ALU.add)
                q3v = q3[:, :].rearrange("p (d o) -> p d o", o=OC)
                q4 = bp.tile([B, 2 * OC], f32, tag="q4", name=f"q4_{t}")
                nc.vector.tensor_tensor(out=q4[:, :].rearrange("p (d o) -> p d o", o=OC),
                                        in0=q3v[:, 0:2, :], in1=q3v[:, 2:4, :], op=ALU.add)
                q4v = q4[:, :].rearrange("p (d o) -> p d o", o=OC)
                n2 = bp.tile([B, OC], f32, tag="n2", name=f"n2_{t}")
                nc.vector.tensor_tensor(out=n2[:, :], in0=q4v[:, 0:1, :].squeeze(1),
                                        in1=q4v[:, 1:2, :].squeeze(1), op=ALU.add)
                # |s| = sqrt(n2) = exp(0.5*ln(n2)); n2=0 -> qq=0 (exp(-inf))
                lnn = bp.tile([B, OC], f32, tag="lnn", name=f"ln_{t}")
                nc.scalar.activation(lnn[:, :], n2[:, :], ACTF.Ln)
                rt = bp.tile([B, OC], f32, tag="rt", name=f"rt_{t}")
                nc.scalar.activation(rt[:, :], lnn[:, :], ACTF.Exp, scale=0.5)
                den = bp.tile([B, OC], f32, tag="den", name=f"den_{t}")
                nc.vector.tensor_scalar_add(den[:, :], n2[:, :], 1.0)
                rec = bp.tile([B, OC], f32, tag="rec", name=f"rec_{t}")
                nc.vector.reciprocal(rec[:, :], den[:, :])
                qq = bp.tile([B, OC], bf16, tag="qq", name=f"qq_{t}")
                nc.vector.tensor_tensor(out=qq[:, :], in0=rt[:, :], in1=rec[:, :],
                                        op=ALU.mult)  # |s|/(1+n2)
                qbc = qq[:, :].unsqueeze(1).broadcast_to([B, OD, OC])
                sv = s_sb[:, :].rearrange("p (d o) -> p d o", o=OC)
                if t == NUM_ROUTES - 1:
                    # write v in (o,d) order so the output DMA is contiguous
                    vt = bp.tile([B, OD2], f32, tag="vtf", name="vt_f")
                    nc.vector.tensor_tensor(
                        out=vt[:, :].rearrange("p (o d) -> p d o", d=OD),
                        in0=sv, in1=qbc, op=ALU.mult)
                    nc.sync.dma_start(out=vout_d[:, :], in_=vt[:, :])
                else:
                    vt = bp.tile([B, OD2], bf16, tag="vtb", name=f"vt{t}")
                    nc.vector.tensor_tensor(
                        out=vt[:, :].rearrange("p (d o) -> p d o", o=OC),
                        in0=sv, in1=qbc, op=ALU.mult)
                    if t == 0:
                        nc.vector.tensor_copy(Vacc[:, :], vt[:, :])
                    else:
                        nc.vector.tensor_add(Vacc[:, :], Vacc[:, :], vt[:, :])
                    for g in range(4):
                        nc.sync.dma_start(out=V4[32 * g:32 * g + 32, :], in_=Vacc[:, :])

            # ======== pass 1: s0 = sum_i u_hat / 64 (dense over (i8,k)) ========
            sacc = pa.tile([128, 512], f32, tag="sacc", name="sacc0")
            for tau in range(NJ):
                for ch in range(4):
                    nc.tensor.matmul(
                        sacc[32 * ch:32 * ch + 32, :],
                        lhsT=x2t[:, tau * B:(tau + 1) * B],
                        rhs=wl[:, tau * OD2 + ch * 512: tau * OD2 + (ch + 1) * 512],
                        start=(tau == 0), stop=(tau == NJ - 1),
                        tile_position=(0, 32 * ch))
            spk = allreduce_s(0, sacc)
            squash(0, spk)

            # ======== passes 2..3: fused agreement/softmax/s, sw-pipelined ====
            for t in range(1, NUM_ROUTES):
                sacc = pa.tile([128, 512], f32, tag="sacc", name=f"sacc{t}")
                uhsb_next = emit_quad(t, 0)          # prologue: quad 0
                tmp_next = emit_mul(t, 0, uhsb_next)
                tmp2_prev = None

                def emit_sel(q, tmp2):
                    for ch in range(4):
                        nc.tensor.matmul(
                            sacc[32 * ch:32 * ch + 32, :], lhsT=sel1[:, :],
                            rhs=tmp2[:, ch * 512:(ch + 1) * 512],
                            start=(q == 0), stop=(q == NQ - 1),
                            tile_position=(0, 32 * ch))

                for q in range(NQ):
                    uhsb, tmp = uhsb_next, tmp_next
                    if q + 1 < NQ:
                        uhsb_next = emit_quad(t, q + 1)
                    # --- PE: fold tmp over d into f32 agr (32 identity MMs) ---
                    agr = pg.tile([128, 64], f32, tag="agr", name=f"agr{t}_{q}")
                    tv = tmp[:, :].rearrange("p (d o) -> p d o", o=OC)
                    for d in range(OD):
                        nc.tensor.matmul(agr[:, 0:64], lhsT=idn[:, :], rhs=tv[:, d, :],
                                         start=(d == 0), stop=(d == OD - 1),
                                         tile_position=(0, 0))
                    # --- PE: selector MMs for q-1 (tmp2 ready -> no PE stall) ---
                    if tmp2_prev is not None:
                        emit_sel(q - 1, tmp2_prev)
                    # --- ACT: softmax numerator + Z straight off PSUM ---
                    eB = sp.tile([128, OC], bf16, tag="eB", name=f"eB{t}_{q}")
                    Zs = sp.tile([128, 1], f32, tag="Zs", name=f"Zs{t}_{q}")
                    nc.scalar.activation(eB[:, :], agr[:, 0:64], ACTF.Exp,
                                         accum_out=Zs[:, :])
                    # --- DVE: mul(q+1) fills the fold/exp gap ---
                    if q + 1 < NQ:
                        tmp_next = emit_mul(t, q + 1, uhsb_next)
                    rZ = sp.tile([128, 1], f32, tag="rZ", name=f"rZ{t}_{q}")
                    nc.vector.reciprocal(rZ[:, :], Zs[:, :])
                    epp = sp.tile([128, OC], bf16, tag="epp", name=f"epp{t}_{q}")
                    nc.vector.tensor_scalar_mul(epp[:, :], eB[:, :], rZ[:, :])
                    # --- DVE: tmp2 = uhsb * c (broadcast over outer d: 2x) ---
                    tmp2 = wp.tile([128, OD2], bf16, tag="tmp2", name=f"tmp2_{t}_{q}")
                    nc.vector.tensor_tensor(
                        out=tmp2[:, :].rearrange("p (d o) -> p d o", o=OC),
                        in0=uhsb[:, :].rearrange("p (d o) -> p d o", o=OC),
                        in1=epp[:, :].unsqueeze(1).broadcast_to([128, OD, OC]),
                        op=ALU.mult)
                    tmp2_prev = tmp2
                emit_sel(NQ - 1, tmp2_prev)
                spk = allreduce_s(t, sacc)
                squash(t, spk)

    nc.compile()
    return nc


def _host_inputs(x, W):
    """Per-core input maps (host-side relayout, not device time)."""
    W0 = np.asarray(W)[0]                       # [IC, OC, OD, KD]
    x = np.asarray(x)                           # [B, IC, KD]
    in_maps = []
    sel1 = np.zeros((128, 32), np.float32)
    for p in range(128):
        sel1[p, p % 32] = 1.0
    idn = np.eye(128, dtype=np.float32)
    for c in range(NCORES):
        # W layout: partition 16*i8 + k, col tau*2048 + d*64 + o  ((d,o) order)
        Wc = (W0[c * ICC:(c + 1) * ICC]
              .reshape(NJ, 8, OC, OD, KD))                      # [tau, i8, o, d, k]
        WL = np.ascontiguousarray(Wc.transpose(1, 4, 0, 3, 2)   # [i8, k, tau, d, o]
                                  ).reshape(128, NJ * OD2)
        xc = x[:, c * ICC:(c + 1) * ICC, :].reshape(B, NJ, 8, KD)   # [b, tau, i8, k]
        xss = []
        for s in range(2):
            Xs = np.zeros((4, 2, KD, NJ, B), np.float32)            # [r, s', k, tau, b]
            Xs[:, s] = xc[:, :, s::2].transpose(2, 3, 1, 0)         # [r, k, tau, b]
            xss.append(Xs.reshape(128, NJ * B))
        X2 = (np.ascontiguousarray(xc.transpose(2, 3, 1, 0))        # [i8, k, tau, b]
              .reshape(128, NJ * B) / float(OC))
        in_maps.append({
            "WL": WL.astype(ml_dtypes.bfloat16),
            "xS0": xss[0].astype(ml_dtypes.bfloat16),
            "xS1": xss[1].astype(ml_dtypes.bfloat16),
            "SEL1": sel1.astype(ml_dtypes.bfloat16),
            "IDN": idn.astype(ml_dtypes.bfloat16),
            "X2": X2.astype(ml_dtypes.bfloat16),
        })
    return in_maps


def kernel(x, W, _want_trace=False):
    from concourse.bass_utils import run_bass_kernel_spmd

    if "nc" not in _CACHE:
        _CACHE["nc"] = _build_program()
    nc = _CACHE["nc"]
    in_maps = _host_inputs(x, W)
    res = run_bass_kernel_spmd(nc, in_maps, core_ids=list(range(NCORES)),
                               trace=_want_trace)
    _CACHE["last_result"] = res
    out = np.asarray(res.results[0]["v_out"], np.float32)
    return out.reshape(B, OC, OD)


# revision 14
# speedup vs baseline: 1.4270x; 1.4270x over previous
"""CapsuleLayer dynamic-routing kernel for 8 Trainium2 NeuronCores. v3

Problem: x[32, 2048, 16], W[1, 2048, 64, 32, 16] -> v[32, 64, 32]
  u_hat = einsum('iodk,bik->biod', W[0], x)
  3 routing iterations (softmax over out_caps, squash over out_dim).

Sharding: in_caps split 8 ways (256/core); W resident in SBUF bf16; s_j
AllReduced per routing pass (only cross-core quantity).

v3 design:
 - columns in (d, o) order, o innermost: the softmax scale e''[p,o]
   broadcasts over the outer d dim keeping step-1 inner -> every wide DVE
   op is a 2x-mode bf16 tensor_tensor (measured ~1.14us per [128,2048]).
 - agreement d-reduction runs on the PE: 32 accumulating identity-matmuls
   (rhs = tmp[:, d, :], lhsT = I) sum the d-slices into an f32 PSUM
   [128,64] tile at 29ns/MM (LDWEIGHTS of the repeated identity pipelines
   through the background weight buffer). Replaces a 2.1us DVE tree.
 - s accumulates in ONE psum bank as [(ch,b), 512] via col-offset
   tile_position selector matmuls (4 col-groups run concurrently, ~330ns).
 - software-pipelined emission: evac(q+1) is queued on ACT before exp(q),
   and mul(q+1) is queued on DVE before recip(q), so neither engine idles
   during the cross-engine ping-pong.
 - no GpSimd elementwise (shares SBUF port with DVE).
 - squash uses sqrt(n2) = exp(0.5*ln(n2)): stays on one ACT table set.
"""

import numpy as np
import ml_dtypes

B, IC, KD, OC, OD = 32, 2048, 16, 64, 32
NCORES = 8
ICC = IC // NCORES                            # 256 in_caps per core
NJ = ICC // 8                                 # 32 tau blocks (8 i per block)
OD2 = OC * OD                                 # 2048 flattened cols, (d, o) order
NQ = 2 * NJ                                   # 64 quads (4 i each)
NUM_ROUTES = 3

_CACHE = {}


def _build_program():
    import concourse.bacc as bacc
    import concourse.tile as tile
    import concourse.mybir as mybir

    f32 = mybir.dt.float32
    bf16 = mybir.dt.bfloat16
    ALU = mybir.AluOpType
    ACTF = mybir.ActivationFunctionType

    nc = bacc.Bacc("TRN2", target_bir_lowering=False, debug=False, num_devices=NCORES)

    WL_d = nc.dram_tensor("WL", [128, NJ * OD2], bf16, kind="ExternalInput").ap()
    xS0_d = nc.dram_tensor("xS0", [128, NJ * B], bf16, kind="ExternalInput").ap()
    xS1_d = nc.dram_tensor("xS1", [128, NJ * B], bf16, kind="ExternalInput").ap()
    SEL1_d = nc.dram_tensor("SEL1", [128, 32], bf16, kind="ExternalInput").ap()
    IDN_d = nc.dram_tensor("IDN", [128, 128], bf16, kind="ExternalInput").ap()
    X2_d = nc.dram_tensor("X2", [128, NJ * B], bf16, kind="ExternalInput").ap()
    vout_d = nc.dram_tensor("v_out", [B, OD2], f32, kind="ExternalOutput").ap()

    with tile.TileContext(nc) as tc:
        with (
            tc.tile_pool(name="const", bufs=1) as cp,
            tc.tile_pool(name="work", bufs=2) as wp,
            tc.tile_pool(name="small", bufs=2) as sp,
            tc.tile_pool(name="bound", bufs=1) as bp,
            tc.tile_pool(name="psum", bufs=2, space="PSUM") as pp,
            tc.tile_pool(name="pagr", bufs=2, space="PSUM") as pg,
            tc.tile_pool(name="psacc", bufs=1, space="PSUM") as pa,
            tc.tile_pool(name="dram", bufs=1, space="DRAM") as dp,
        ):
            # ---- resident inputs ----
            wl = cp.tile([128, NJ * OD2], bf16, tag="wl")
            for blk in range(8):
                w = NJ * OD2 // 8
                nc.sync.dma_start(out=wl[:, blk * w:(blk + 1) * w],
                                  in_=WL_d[:, blk * w:(blk + 1) * w])
            xs = [cp.tile([128, NJ * B], bf16, tag=f"xs{s}", name=f"xs{s}") for s in range(2)]
            nc.sync.dma_start(out=xs[0][:, :], in_=xS0_d[:, :])
            nc.sync.dma_start(out=xs[1][:, :], in_=xS1_d[:, :])
            sel1 = cp.tile([128, 32], bf16, tag="sel1")
            nc.sync.dma_start(out=sel1[:, :], in_=SEL1_d[:, :])
            idn = cp.tile([128, 128], bf16, tag="idn")
            nc.sync.dma_start(out=idn[:, :], in_=IDN_d[:, :])
            x2t = cp.tile([128, NJ * B], bf16, tag="x2t")
            nc.sync.dma_start(out=x2t[:, :], in_=X2_d[:, :])

            # ---- persistent state ----
            V4 = cp.tile([128, OD2], bf16, tag="V4")    # Vacc replicated x4 part-groups
            Vacc = cp.tile([B, OD2], bf16, tag="Vacc")  # running sum of v_t, (d,o) cols

            ar_in = [dp.tile([128, 512], f32, tag=f"ari{t}", name=f"ari{t}") for t in range(NUM_ROUTES)]
            ar_out = [dp.tile([128, 512], f32, tag=f"aro{t}", name=f"aro{t}") for t in range(NUM_ROUTES)]

            def emit_quad(t, q):
                """PE u_hat quad q + ACT evacuation -> uhsb (bf16, (d,o))."""
                jj, s_ = divmod(q, 2)
                uhp = [pp.tile([128, 1024], f32, tag="uhp", name=f"uhp{t}_{q}_{h}")
                       for h in range(2)]
                for h in range(2):
                    for ch in range(2):
                        col = jj * OD2 + (2 * h + ch) * 512
                        for r in range(4):
                            nc.tensor.matmul(
                                uhp[h][32 * r:32 * r + 32, ch * 512:(ch + 1) * 512],
                                lhsT=xs[s_][32 * r:32 * r + 32, jj * B:(jj + 1) * B],
                                rhs=wl[32 * r:32 * r + 32, col: col + 512],
                                start=True, stop=True,
                                tile_position=(32 * r, 32 * r),
                            )
                uhsb = wp.tile([128, OD2], bf16, tag="uhb", name=f"uhsb{t}_{q}",
                               bufs=3)
                for h in range(2):
                    nc.scalar.copy(uhsb[:, h * 1024:(h + 1) * 1024], uhp[h][:, :])
                return uhsb

            def emit_mul(t, q, uhsb):
                """DVE tmp = uhsb * V4 (bf16 2x)."""
                tmp = wp.tile([128, OD2], bf16, tag="tmp", name=f"tmp{t}_{q}")
                nc.vector.tensor_tensor(out=tmp[:, :], in0=uhsb[:, :], in1=V4[:, :],
                                        op=ALU.mult)
                return tmp

            def allreduce_s(t, src_psum):
                """Evacuate packed s (psum [128,512] f32) -> allreduce."""
                s_sb = cp.tile([128, 512], f32, tag="ssb", name=f"s_sb{t}")
                nc.scalar.copy(s_sb[:, :], src_psum[:, :])
                nc.sync.dma_start(out=ar_in[t][:, :], in_=s_sb[:, :])
                nc.gpsimd.collective_compute(
                    "AllReduce", ALU.add,
                    replica_groups=[list(range(NCORES))],
                    ins=[ar_in[t].opt()],
                    outs=[ar_out[t].opt()],
                )
                nc.sync.dma_start(out=s_sb[:, :], in_=ar_out[t][:, :])
                # unpack [(ch,b), 512] -> [32, 2048]
                spk = bp.tile([B, OD2], f32, tag="spk", name=f"spk{t}")
                for ch in range(4):
                    nc.sync.dma_start(out=spk[:, ch * 512:(ch + 1) * 512],
                                      in_=s_sb[32 * ch:32 * ch + 32, :])
                return spk

            def squash(t, s_sb):
                """v_t = squash(s_sb [32,2048] f32, (d,o) cols)."""
                sq = bp.tile([B, OD2], bf16, tag="sqv", name=f"sq{t}")
                nc.scalar.activation(sq[:, :], s_sb[:, :], ACTF.Square)
                sqv = sq[:, :].rearrange("p (d o) -> p d o", o=OC)
                q1 = bp.tile([B, 16 * OC], bf16, tag="q1", name=f"q1_{t}")
                nc.vector.tensor_tensor(out=q1[:, :].rearrange("p (d o) -> p d o", o=OC),
                                        in0=sqv[:, 0:16, :], in1=sqv[:, 16:32, :], op=ALU.add)
                q1v = q1[:, :].rearrange("p (d o) -> p d o", o=OC)
                q2 = bp.tile([B, 8 * OC], bf16, tag="q2", name=f"q2_{t}")
                nc.vector.tensor_tensor(out=q2[:, :].rearrange("p (d o) -> p d o", o=OC),
                                        in0=q1v[:, 0:8, :], in1=q1v[:, 8:16, :], op=ALU.add)
                q2v = q2[:, :].rearrange("p (d o) -> p d o", o=OC)
                q3 = bp.tile([B, 4 * OC], bf16, tag="q3", name=f"q3_{t}")
                nc.vector.tensor_tensor(out=q3[:, :].rearrange("p (d o) -> p d o", o=OC),
                                        in0=q2v[:, 0:4, :], in1=q2v[:, 4:8, :], op=ALU.add)
                q3v = q3[:, :].rearrange("p (d o) -> p d o", o=OC)
                q4 = bp.tile([B, 2 * OC], f32, tag="q4", name=f"q4_{t}")
                nc.vector.tensor_tensor(out=q4[:, :].rearrange("p (d o) -> p d o", o=OC),
                                        in0=q3v[:, 0:2, :], in1=q3v[:, 2:4, :], op=ALU.add)
                q4v = q4[:, :].rearrange("p (d o) -> p d o", o=OC)
                n2 = bp.tile([B, OC], f32, tag="n2", name=f"n2_{t}")
                nc.vector.tensor_tensor(out=n2[:, :], in0=q4v[:, 0:1, :].squeeze(1),
                                        in1=q4v[:, 1:2, :].squeeze(1), op=ALU.add)
                # |s| = sqrt(n2) = exp(0.5*ln(n2)); n2=0 -> qq=0 (exp(-inf))
                lnn = bp.tile([B, OC], f32, tag="lnn", name=f"ln_{t}")
                nc.scalar.activation(lnn[:, :], n2[:, :], ACTF.Ln)
                rt = bp.tile([B, OC], f32, tag="rt", name=f"rt_{t}")
                nc.scalar.activation(rt[:, :], lnn[:, :], ACTF.Exp, scale=0.5)
                den = bp.tile([B, OC], f32, tag="den", name=f"den_{t}")
                nc.vector.tensor_scalar_add(den[:, :], n2[:, :], 1.0)
                rec = bp.tile([B, OC], f32, tag="rec", name=f"rec_{t}")
                nc.vector.reciprocal(rec[:, :], den[:, :])
                qq = bp.tile([B, OC], bf16, tag="qq", name=f"qq_{t}")
                nc.vector.tensor_tensor(out=qq[:, :], in0=rt[:, :], in1=rec[:, :],
                                        op=ALU.mult)  # |s|/(1+n2)
                qbc = qq[:, :].unsqueeze(1).broadcast_to([B, OD, OC])
                sv = s_sb[:, :].rearrange("p (d o) -> p d o", o=OC)
                if t == NUM_ROUTES - 1:
                    # write v in (o,d) order so the output DMA is contiguous
                    vt = bp.tile([B, OD2], f32, tag="vtf", name="vt_f")
                    nc.vector.tensor_tensor(
                        out=vt[:, :].rearrange("p (o d) -> p d o", d=OD),
                        in0=sv, in1=qbc, op=ALU.mult)
                    nc.sync.dma_start(out=vout_d[:, :], in_=vt[:, :])
                else:
                    vt = bp.tile([B, OD2], bf16, tag="vtb", name=f"vt{t}")
                    nc.vector.tensor_tensor(
                        out=vt[:, :].rearrange("p (d o) -> p d o", o=OC),
                        in0=sv, in1=qbc, op=ALU.mult)
                    if t == 0:
                        nc.vector.tensor_copy(Vacc[:, :], vt[:, :])
                    else:
                        nc.vector.tensor_add(Vacc[:, :], Vacc[:, :], vt[:, :])
                    for g in range(4):
                        nc.sync.dma_start(out=V4[32 * g:32 * g + 32, :], in_=Vacc[:, :])

            # ======== pass 1: s0 = sum_i u_hat / 64 (dense over (i8,k)) ========
            sacc = pa.tile([128, 512], f32, tag="sacc", name="sacc0")
            for tau in range(NJ):
                for ch in range(4):
                    nc.tensor.matmul(
                        sacc[32 * ch:32 * ch + 32, :],
                        lhsT=x2t[:, tau * B:(tau + 1) * B],
                        rhs=wl[:, tau * OD2 + ch * 512: tau * OD2 + (ch + 1) * 512],
                        start=(tau == 0), stop=(tau == NJ - 1),
                        tile_position=(0, 32 * ch))
            spk = allreduce_s(0, sacc)
            squash(0, spk)

            # ======== passes 2..3: fused agreement/softmax/s, sw-pipelined ====
            for t in range(1, NUM_ROUTES):
                sacc = pa.tile([128, 512], f32, tag="sacc", name=f"sacc{t}")
                uhsb_next = emit_quad(t, 0)          # prologue: quad 0
                tmp_next = emit_mul(t, 0, uhsb_next)
                tmp2_prev = None

                def emit_sel(q, tmp2):
                    for ch in range(4):
                        nc.tensor.matmul(
                            sacc[32 * ch:32 * ch + 32, :], lhsT=sel1[:, :],
                            rhs=tmp2[:, ch * 512:(ch + 1) * 512],
                            start=(q == 0), stop=(q == NQ - 1),
                            tile_position=(0, 32 * ch))

                for q in range(NQ):
                    uhsb, tmp = uhsb_next, tmp_next
                    if q + 1 < NQ:
                        uhsb_next = emit_quad(t, q + 1)
                    # --- PE: fold tmp over d into f32 agr (32 identity MMs) ---
                    agr = pg.tile([128, 64], f32, tag="agr", name=f"agr{t}_{q}")
                    tv = tmp[:, :].rearrange("p (d o) -> p d o", o=OC)
                    for d in range(OD):
                        nc.tensor.matmul(agr[:, 0:64], lhsT=idn[:, :], rhs=tv[:, d, :],
                                         start=(d == 0), stop=(d == OD - 1),
                                         tile_position=(0, 0))
                    # --- PE: selector MMs for q-1 (tmp2 ready -> no PE stall) ---
                    if tmp2_prev is not None:
                        emit_sel(q - 1, tmp2_prev)
                    # --- ACT: softmax numerator + Z straight off PSUM ---
                    eB = sp.tile([128, OC], bf16, tag="eB", name=f"eB{t}_{q}")
                    Zs = sp.tile([128, 1], f32, tag="Zs", name=f"Zs{t}_{q}")
                    nc.scalar.activation(eB[:, :], agr[:, 0:64], ACTF.Exp,
                                         accum_out=Zs[:, :])
                    # --- DVE: mul(q+1) fills the fold/exp gap ---
                    if q + 1 < NQ:
                        tmp_next = emit_mul(t, q + 1, uhsb_next)
                    rZ = sp.tile([128, 1], f32, tag="rZ", name=f"rZ{t}_{q}")
                    nc.vector.reciprocal(rZ[:, :], Zs[:, :])
                    epp = sp.tile([128, OC], bf16, tag="epp", name=f"epp{t}_{q}")
                    nc.vector.tensor_scalar_mul(epp[:, :], eB[:, :], rZ[:, :])
                    # --- DVE: tmp2 = uhsb * c (broadcast over outer d: 2x) ---
                    tmp2 = wp.tile([128, OD2], bf16, tag="tmp2", name=f"tmp2_{t}_{q}")
                    nc.vector.tensor_tensor(
                        out=tmp2[:, :].rearrange("p (d o) -> p d o", o=OC),
                        in0=uhsb[:, :].rearrange("p (d o) -> p d o", o=OC),
                        in1=epp[:, :].unsqueeze(1).broadcast_to([128, OD, OC]),
                        op=ALU.mult)
                    tmp2_prev = tmp2
                emit_sel(NQ - 1, tmp2_prev)
                spk = allreduce_s(t, sacc)
                squash(t, spk)

    nc.compile()
    return nc


def _host_inputs(x, W):
    """Per-core input maps (host-side relayout, not device time)."""
    W0 = np.asarray(W)[0]                       # [IC, OC, OD, KD]
    x = np.asarray(x)                           # [B, IC, KD]
    in_maps = []
    sel1 = np.zeros((128, 32), np.float32)
    for p in range(128):
        sel1[p, p % 32] = 1.0
    idn = np.eye(128, dtype=np.float32)
    for c in range(NCORES):
        # W layout: partition 16*i8 + k, col tau*2048 + d*64 + o  ((d,o) order)
        Wc = (W0[c * ICC:(c + 1) * ICC]
              .reshape(NJ, 8, OC, OD, KD))                      # [tau, i8, o, d, k]
        WL = np.ascontiguousarray(Wc.transpose(1, 4, 0, 3, 2)   # [i8, k, tau, d, o]
                                  ).reshape(128, NJ * OD2)
        xc = x[:, c * ICC:(c + 1) * ICC, :].reshape(B, NJ, 8, KD)   # [b, tau, i8, k]
        xss = []
        for s in range(2):
            Xs = np.zeros((4, 2, KD, NJ, B), np.float32)            # [r, s', k, tau, b]
            Xs[:, s] = xc[:, :, s::2].transpose(2, 3, 1, 0)         # [r, k, tau, b]
            xss.append(Xs.reshape(128, NJ * B))
        X2 = (np.ascontiguousarray(xc.transpose(2, 3, 1, 0))        # [i8, k, tau, b]
              .reshape(128, NJ * B) / float(OC))
        in_maps.append({
            "WL": WL.astype(ml_dtypes.bfloat16),
            "xS0": xss[0].astype(ml_dtypes.bfloat16),
            "xS1": xss[1].astype(ml_dtypes.bfloat16),
            "SEL1": sel1.astype(ml_dtypes.bfloat16),
            "IDN": idn.astype(ml_dtypes.bfloat16),
            "X2": X2.astype(ml_dtypes.bfloat16),
        })
    return in_maps


def kernel(x, W, _want_trace=False):
    from concourse.bass_utils import run_bass_kernel_spmd

    if "nc" not in _CACHE:
        _CACHE["nc"] = _build_program()
    nc = _CACHE["nc"]
    in_maps = _host_inputs(x, W)
    res = run_bass_kernel_spmd(nc, in_maps, core_ids=list(range(NCORES)),
                               trace=_want_trace)
    _CACHE["last_result"] = res
    out = np.asarray(res.results[0]["v_out"], np.float32)
    return out.reshape(B, OC, OD)


# revision 15
# speedup vs baseline: 1.5084x; 1.0571x over previous
"""CapsuleLayer dynamic-routing kernel for 8 Trainium2 NeuronCores. v3

Problem: x[32, 2048, 16], W[1, 2048, 64, 32, 16] -> v[32, 64, 32]
  u_hat = einsum('iodk,bik->biod', W[0], x)
  3 routing iterations (softmax over out_caps, squash over out_dim).

Sharding: in_caps split 8 ways (256/core); W resident in SBUF bf16; s_j
AllReduced per routing pass (only cross-core quantity).

v3 design:
 - columns in (d, o) order, o innermost: the softmax scale e''[p,o]
   broadcasts over the outer d dim keeping step-1 inner -> every wide DVE
   op is a 2x-mode bf16 tensor_tensor (measured ~1.14us per [128,2048]).
 - agreement d-reduction runs on the PE: 32 accumulating identity-matmuls
   (rhs = tmp[:, d, :], lhsT = I) sum the d-slices into an f32 PSUM
   [128,64] tile at 29ns/MM (LDWEIGHTS of the repeated identity pipelines
   through the background weight buffer). Replaces a 2.1us DVE tree.
 - s accumulates in ONE psum bank as [(ch,b), 512] via col-offset
   tile_position selector matmuls (4 col-groups run concurrently, ~330ns).
 - software-pipelined emission: evac(q+1) is queued on ACT before exp(q),
   and mul(q+1) is queued on DVE before recip(q), so neither engine idles
   during the cross-engine ping-pong.
 - no GpSimd elementwise (shares SBUF port with DVE).
 - squash uses sqrt(n2) = exp(0.5*ln(n2)): stays on one ACT table set.
"""

import numpy as np
import ml_dtypes

B, IC, KD, OC, OD = 32, 2048, 16, 64, 32
NCORES = 8
ICC = IC // NCORES                            # 256 in_caps per core
NJ = ICC // 8                                 # 32 tau blocks (8 i per block)
OD2 = OC * OD                                 # 2048 flattened cols, (d, o) order
NQ = 2 * NJ                                   # 64 quads (4 i each)
NUM_ROUTES = 3

_CACHE = {}


def _build_program():
    import concourse.bacc as bacc
    import concourse.tile as tile
    import concourse.mybir as mybir

    f32 = mybir.dt.float32
    bf16 = mybir.dt.bfloat16
    ALU = mybir.AluOpType
    ACTF = mybir.ActivationFunctionType

    nc = bacc.Bacc("TRN2", target_bir_lowering=False, debug=False, num_devices=NCORES)

    WL_d = nc.dram_tensor("WL", [128, NJ * OD2], bf16, kind="ExternalInput").ap()
    xS0_d = nc.dram_tensor("xS0", [128, NJ * B], bf16, kind="ExternalInput").ap()
    xS1_d = nc.dram_tensor("xS1", [128, NJ * B], bf16, kind="ExternalInput").ap()
    SEL1_d = nc.dram_tensor("SEL1", [128, 32], bf16, kind="ExternalInput").ap()
    IDN_d = nc.dram_tensor("IDN", [128, 128], bf16, kind="ExternalInput").ap()
    X2_d = nc.dram_tensor("X2", [128, NJ * B], bf16, kind="ExternalInput").ap()
    vout_d = nc.dram_tensor("v_out", [B, OD2], f32, kind="ExternalOutput").ap()

    with tile.TileContext(nc) as tc:
        with (
            tc.tile_pool(name="const", bufs=1) as cp,
            tc.tile_pool(name="work", bufs=2) as wp,
            tc.tile_pool(name="small", bufs=2) as sp,
            tc.tile_pool(name="bound", bufs=1) as bp,
            tc.tile_pool(name="psum", bufs=2, space="PSUM") as pp,
            tc.tile_pool(name="pagr", bufs=2, space="PSUM") as pg,
            tc.tile_pool(name="psacc", bufs=1, space="PSUM") as pa,
            tc.tile_pool(name="dram", bufs=1, space="DRAM") as dp,
        ):
            # ---- resident inputs ----
            wl = cp.tile([128, NJ * OD2], bf16, tag="wl")
            for blk in range(16):
                w = NJ * OD2 // 16
                nc.sync.dma_start(out=wl[:, blk * w:(blk + 1) * w],
                                  in_=WL_d[:, blk * w:(blk + 1) * w])
            xs = [cp.tile([128, NJ * B], bf16, tag=f"xs{s}", name=f"xs{s}") for s in range(2)]
            nc.sync.dma_start(out=xs[0][:, :], in_=xS0_d[:, :])
            nc.sync.dma_start(out=xs[1][:, :], in_=xS1_d[:, :])
            sel1 = cp.tile([128, 32], bf16, tag="sel1")
            nc.sync.dma_start(out=sel1[:, :], in_=SEL1_d[:, :])
            idn = cp.tile([128, 128], bf16, tag="idn")
            nc.sync.dma_start(out=idn[:, :], in_=IDN_d[:, :])
            x2t = cp.tile([128, NJ * B], bf16, tag="x2t")
            nc.sync.dma_start(out=x2t[:, :], in_=X2_d[:, :])

            # ---- persistent state ----
            V4 = cp.tile([128, OD2], bf16, tag="V4")    # Vacc replicated x4 part-groups
            Vacc = cp.tile([B, OD2], bf16, tag="Vacc")  # running sum of v_t, (d,o) cols

            ar_in = [dp.tile([128, 512], bf16, tag=f"ari{t}", name=f"ari{t}") for t in range(NUM_ROUTES)]
            ar_out = [dp.tile([128, 512], bf16, tag=f"aro{t}", name=f"aro{t}") for t in range(NUM_ROUTES)]
            arw_in = dp.tile([128, 8], bf16, tag="arwi")
            arw_out = dp.tile([128, 8], bf16, tag="arwo")
            # warm up the collective rings while WL streams in
            warmt = cp.tile([128, 8], bf16, tag="warmt")
            nc.vector.memset(warmt[:, :], 0.0)
            nc.sync.dma_start(out=arw_in[:, :], in_=warmt[:, :])
            nc.gpsimd.collective_compute(
                "AllReduce", ALU.add,
                replica_groups=[list(range(NCORES))],
                ins=[arw_in.opt()], outs=[arw_out.opt()],
            )

            def emit_quad(t, q):
                """PE u_hat quad q + ACT evacuation -> uhsb (bf16, (d,o))."""
                jj, s_ = divmod(q, 2)
                uhp = [pp.tile([128, 1024], f32, tag="uhp", name=f"uhp{t}_{q}_{h}")
                       for h in range(2)]
                for h in range(2):
                    for ch in range(2):
                        col = jj * OD2 + (2 * h + ch) * 512
                        for r in range(4):
                            nc.tensor.matmul(
                                uhp[h][32 * r:32 * r + 32, ch * 512:(ch + 1) * 512],
                                lhsT=xs[s_][32 * r:32 * r + 32, jj * B:(jj + 1) * B],
                                rhs=wl[32 * r:32 * r + 32, col: col + 512],
                                start=True, stop=True,
                                tile_position=(32 * r, 32 * r),
                            )
                uhsb = wp.tile([128, OD2], bf16, tag="uhb", name=f"uhsb{t}_{q}",
                               bufs=4)
                for h in range(2):
                    nc.scalar.copy(uhsb[:, h * 1024:(h + 1) * 1024], uhp[h][:, :])
                return uhsb

            def emit_mul(t, q, uhsb):
                """DVE tmp = uhsb * V4 (bf16 2x)."""
                tmp = wp.tile([128, OD2], bf16, tag="tmp", name=f"tmp{t}_{q}")
                nc.vector.tensor_tensor(out=tmp[:, :], in0=uhsb[:, :], in1=V4[:, :],
                                        op=ALU.mult)
                return tmp

            def allreduce_s(t, src_psum):
                """Evacuate packed s (psum [128,512] f32) -> bf16 allreduce."""
                s_sb = cp.tile([128, 512], bf16, tag="ssb", name=f"s_sb{t}")
                nc.scalar.copy(s_sb[:, :], src_psum[:, :])
                nc.sync.dma_start(out=ar_in[t][:, :], in_=s_sb[:, :])
                nc.gpsimd.collective_compute(
                    "AllReduce", ALU.add,
                    replica_groups=[list(range(NCORES))],
                    ins=[ar_in[t].opt()],
                    outs=[ar_out[t].opt()],
                )
                # one strided DMA: dram [(ch,b), 512] -> sbuf [32, (ch,512)]
                spk = bp.tile([B, OD2], bf16, tag="spk", name=f"spk{t}")
                nc.sync.dma_start(
                    out=spk[:, :].rearrange("p (c j) -> p c j", c=4),
                    in_=ar_out[t][:, :].rearrange("(c p) j -> p c j", c=4))
                return spk

            def squash(t, s_sb):
                """v_t = squash(s_sb [32,2048] f32, (d,o) cols)."""
                sq = bp.tile([B, OD2], bf16, tag="sqv", name=f"sq{t}")
                nc.scalar.activation(sq[:, :], s_sb[:, :], ACTF.Square)
                sqv = sq[:, :].rearrange("p (d o) -> p d o", o=OC)
                q1 = bp.tile([B, 16 * OC], bf16, tag="q1", name=f"q1_{t}")
                nc.vector.tensor_tensor(out=q1[:, :].rearrange("p (d o) -> p d o", o=OC),
                                        in0=sqv[:, 0:16, :], in1=sqv[:, 16:32, :], op=ALU.add)
                q1v = q1[:, :].rearrange("p (d o) -> p d o", o=OC)
                q2 = bp.tile([B, 8 * OC], bf16, tag="q2", name=f"q2_{t}")
                nc.vector.tensor_tensor(out=q2[:, :].rearrange("p (d o) -> p d o", o=OC),
                                        in0=q1v[:, 0:8, :], in1=q1v[:, 8:16, :], op=ALU.add)
                q2v = q2[:, :].rearrange("p (d o) -> p d o", o=OC)
                q3 = bp.tile([B, 4 * OC], bf16, tag="q3", name=f"q3_{t}")
                nc.vector.tensor_tensor(out=q3[:, :].rearrange("p (d o) -> p d o", o=OC),
                                        in0=q2v[:, 0:4, :], in1=q2v[:, 4:8, :], op=ALU.add)
                q3v = q3[:, :].rearrange("p (d o) -> p d o", o=OC)
                q4 = bp.tile([B, 2 * OC], f32, tag="q4", name=f"q4_{t}")
                nc.vector.tensor_tensor(out=q4[:, :].rearrange("p (d o) -> p d o", o=OC),
                                        in0=q3v[:, 0:2, :], in1=q3v[:, 2:4, :], op=ALU.add)
                q4v = q4[:, :].rearrange("p (d o) -> p d o", o=OC)
                n2 = bp.tile([B, OC], f32, tag="n2", name=f"n2_{t}")
                nc.vector.tensor_tensor(out=n2[:, :], in0=q4v[:, 0:1, :].squeeze(1),
                                        in1=q4v[:, 1:2, :].squeeze(1), op=ALU.add)
                # |s| = sqrt(n2) = exp(0.5*ln(n2)); n2=0 -> qq=0 (exp(-inf))
                lnn = bp.tile([B, OC], f32, tag="lnn", name=f"ln_{t}")
                nc.scalar.activation(lnn[:, :], n2[:, :], ACTF.Ln)
                rt = bp.tile([B, OC], f32, tag="rt", name=f"rt_{t}")
                nc.scalar.activation(rt[:, :], lnn[:, :], ACTF.Exp, scale=0.5)
                den = bp.tile([B, OC], f32, tag="den", name=f"den_{t}")
                nc.vector.tensor_scalar_add(den[:, :], n2[:, :], 1.0)
                rec = bp.tile([B, OC], f32, tag="rec", name=f"rec_{t}")
                nc.vector.reciprocal(rec[:, :], den[:, :])
                qq = bp.tile([B, OC], bf16, tag="qq", name=f"qq_{t}")
                nc.vector.tensor_tensor(out=qq[:, :], in0=rt[:, :], in1=rec[:, :],
                                        op=ALU.mult)  # |s|/(1+n2)
                qbc = qq[:, :].unsqueeze(1).broadcast_to([B, OD, OC])
                sv = s_sb[:, :].rearrange("p (d o) -> p d o", o=OC)
                if t == NUM_ROUTES - 1:
                    # write v in (o,d) order so the output DMA is contiguous
                    vt = bp.tile([B, OD2], f32, tag="vtf", name="vt_f")
                    nc.vector.tensor_tensor(
                        out=vt[:, :].rearrange("p (o d) -> p d o", d=OD),
                        in0=sv, in1=qbc, op=ALU.mult)
                    nc.sync.dma_start(out=vout_d[:, :], in_=vt[:, :])
                else:
                    vt = bp.tile([B, OD2], bf16, tag="vtb", name=f"vt{t}")
                    nc.vector.tensor_tensor(
                        out=vt[:, :].rearrange("p (d o) -> p d o", o=OC),
                        in0=sv, in1=qbc, op=ALU.mult)
                    if t == 0:
                        nc.vector.tensor_copy(Vacc[:, :], vt[:, :])
                    else:
                        nc.vector.tensor_add(Vacc[:, :], Vacc[:, :], vt[:, :])
                    for g in range(4):
                        nc.sync.dma_start(out=V4[32 * g:32 * g + 32, :], in_=Vacc[:, :])

            # ======== pass 1: s0 = sum_i u_hat / 64 (dense over (i8,k)) ========
            sacc = pa.tile([128, 512], f32, tag="sacc", name="sacc0")
            for tau in range(NJ):
                for ch in range(4):
                    nc.tensor.matmul(
                        sacc[32 * ch:32 * ch + 32, :],
                        lhsT=x2t[:, tau * B:(tau + 1) * B],
                        rhs=wl[:, tau * OD2 + ch * 512: tau * OD2 + (ch + 1) * 512],
                        start=(tau == 0), stop=(tau == NJ - 1),
                        tile_position=(0, 32 * ch))
            spk = allreduce_s(0, sacc)
            squash(0, spk)

            # ======== passes 2..3: fused agreement/softmax/s, sw-pipelined ====
            for t in range(1, NUM_ROUTES):
                sacc = pa.tile([128, 512], f32, tag="sacc", name=f"sacc{t}")
                uhsb_next = emit_quad(t, 0)          # prologue: quad 0
                tmp_next = emit_mul(t, 0, uhsb_next)
                tmp2_prev = None

                def emit_sel(q, tmp2):
                    for ch in range(4):
                        nc.tensor.matmul(
                            sacc[32 * ch:32 * ch + 32, :], lhsT=sel1[:, :],
                            rhs=tmp2[:, ch * 512:(ch + 1) * 512],
                            start=(q == 0), stop=(q == NQ - 1),
                            tile_position=(0, 32 * ch))

                for q in range(NQ):
                    uhsb, tmp = uhsb_next, tmp_next
                    if q + 1 < NQ:
                        uhsb_next = emit_quad(t, q + 1)
                    # --- PE: fold tmp over d into f32 agr (32 identity MMs) ---
                    agr = pg.tile([128, 64], f32, tag="agr", name=f"agr{t}_{q}")
                    tv = tmp[:, :].rearrange("p (d o) -> p d o", o=OC)
                    for d in range(OD):
                        nc.tensor.matmul(agr[:, 0:64], lhsT=idn[:, :], rhs=tv[:, d, :],
                                         start=(d == 0), stop=(d == OD - 1),
                                         tile_position=(0, 0))
                    # --- PE: selector MMs for q-1 (tmp2 ready -> no PE stall) ---
                    if tmp2_prev is not None:
                        emit_sel(q - 1, tmp2_prev)
                    # --- ACT: softmax numerator + Z straight off PSUM ---
                    eB = sp.tile([128, OC], bf16, tag="eB", name=f"eB{t}_{q}")
                    Zs = sp.tile([128, 1], f32, tag="Zs", name=f"Zs{t}_{q}")
                    nc.scalar.activation(eB[:, :], agr[:, 0:64], ACTF.Exp,
                                         accum_out=Zs[:, :])
                    # --- DVE: mul(q+1) fills the fold/exp gap ---
                    if q + 1 < NQ:
                        tmp_next = emit_mul(t, q + 1, uhsb_next)
                    rZ = sp.tile([128, 1], f32, tag="rZ", name=f"rZ{t}_{q}")
                    nc.vector.reciprocal(rZ[:, :], Zs[:, :])
                    epp = sp.tile([128, OC], bf16, tag="epp", name=f"epp{t}_{q}")
                    nc.vector.tensor_scalar_mul(epp[:, :], eB[:, :], rZ[:, :])
                    # --- DVE: tmp2 = uhsb * c (broadcast over outer d: 2x) ---
                    tmp2 = wp.tile([128, OD2], bf16, tag="tmp2", name=f"tmp2_{t}_{q}")
                    nc.vector.tensor_tensor(
                        out=tmp2[:, :].rearrange("p (d o) -> p d o", o=OC),
                        in0=uhsb[:, :].rearrange("p (d o) -> p d o", o=OC),
                        in1=epp[:, :].unsqueeze(1).broadcast_to([128, OD, OC]),
                        op=ALU.mult)
                    tmp2_prev = tmp2
                emit_sel(NQ - 1, tmp2_prev)
                spk = allreduce_s(t, sacc)
                squash(t, spk)

    nc.compile()
    return nc


def _host_inputs(x, W):
    """Per-core input maps (host-side relayout, not device time)."""
    W0 = np.asarray(W)[0]                       # [IC, OC, OD, KD]
    x = np.asarray(x)                           # [B, IC, KD]
    in_maps = []
    sel1 = np.zeros((128, 32), np.float32)
    for p in range(128):
        sel1[p, p % 32] = 1.0
    idn = np.eye(128, dtype=np.float32)
    for c in range(NCORES):
        # W layout: partition 16*i8 + k, col tau*2048 + d*64 + o  ((d,o) order)
        Wc = (W0[c * ICC:(c + 1) * ICC]
              .reshape(NJ, 8, OC, OD, KD))                      # [tau, i8, o, d, k]
        WL = np.ascontiguousarray(Wc.transpose(1, 4, 0, 3, 2)   # [i8, k, tau, d, o]
                                  ).reshape(128, NJ * OD2)
        xc = x[:, c * ICC:(c + 1) * ICC, :].reshape(B, NJ, 8, KD)   # [b, tau, i8, k]
        xss = []
        for s in range(2):
            Xs = np.zeros((4, 2, KD, NJ, B), np.float32)            # [r, s', k, tau, b]
            Xs[:, s] = xc[:, :, s::2].transpose(2, 3, 1, 0)         # [r, k, tau, b]
            xss.append(Xs.reshape(128, NJ * B))
        X2 = (np.ascontiguousarray(xc.transpose(2, 3, 1, 0))        # [i8, k, tau, b]
              .reshape(128, NJ * B) / float(OC))
        in_maps.append({
            "WL": WL.astype(ml_dtypes.bfloat16),
            "xS0": xss[0].astype(ml_dtypes.bfloat16),
            "xS1": xss[1].astype(ml_dtypes.bfloat16),
            "SEL1": sel1.astype(ml_dtypes.bfloat16),
            "IDN": idn.astype(ml_dtypes.bfloat16),
            "X2": X2.astype(ml_dtypes.bfloat16),
        })
    return in_maps


def kernel(x, W, _want_trace=False):
    from concourse.bass_utils import run_bass_kernel_spmd

    if "nc" not in _CACHE:
        _CACHE["nc"] = _build_program()
    nc = _CACHE["nc"]
    in_maps = _host_inputs(x, W)
    res = run_bass_kernel_spmd(nc, in_maps, core_ids=list(range(NCORES)),
                               trace=_want_trace)
    _CACHE["last_result"] = res
    out = np.asarray(res.results[0]["v_out"], np.float32)
    return out.reshape(B, OC, OD)


# revision 18
# speedup vs baseline: 1.5146x; 1.0041x over previous
"""CapsuleLayer dynamic-routing kernel for 8 Trainium2 NeuronCores. v3

Problem: x[32, 2048, 16], W[1, 2048, 64, 32, 16] -> v[32, 64, 32]
  u_hat = einsum('iodk,bik->biod', W[0], x)
  3 routing iterations (softmax over out_caps, squash over out_dim).

Sharding: in_caps split 8 ways (256/core); W resident in SBUF bf16; s_j
AllReduced per routing pass (only cross-core quantity).

v3 design:
 - columns in (d, o) order, o innermost: the softmax scale e''[p,o]
   broadcasts over the outer d dim keeping step-1 inner -> every wide DVE
   op is a 2x-mode bf16 tensor_tensor (measured ~1.14us per [128,2048]).
 - agreement d-reduction runs on the PE: 32 accumulating identity-matmuls
   (rhs = tmp[:, d, :], lhsT = I) sum the d-slices into an f32 PSUM
   [128,64] tile at 29ns/MM (LDWEIGHTS of the repeated identity pipelines
   through the background weight buffer). Replaces a 2.1us DVE tree.
 - s accumulates in ONE psum bank as [(ch,b), 512] via col-offset
   tile_position selector matmuls (4 col-groups run concurrently, ~330ns).
 - software-pipelined emission: evac(q+1) is queued on ACT before exp(q),
   and mul(q+1) is queued on DVE before recip(q), so neither engine idles
   during the cross-engine ping-pong.
 - no GpSimd elementwise (shares SBUF port with DVE).
 - squash uses sqrt(n2) = exp(0.5*ln(n2)): stays on one ACT table set.
"""

import numpy as np
import ml_dtypes

B, IC, KD, OC, OD = 32, 2048, 16, 64, 32
NCORES = 8
ICC = IC // NCORES                            # 256 in_caps per core
NJ = ICC // 8                                 # 32 tau blocks (8 i per block)
OD2 = OC * OD                                 # 2048 flattened cols, (d, o) order
NQ = 2 * NJ                                   # 64 quads (4 i each)
NUM_ROUTES = 3

_CACHE = {}


def _build_program():
    import concourse.bacc as bacc
    import concourse.tile as tile
    import concourse.mybir as mybir

    f32 = mybir.dt.float32
    bf16 = mybir.dt.bfloat16
    ALU = mybir.AluOpType
    ACTF = mybir.ActivationFunctionType

    nc = bacc.Bacc("TRN2", target_bir_lowering=False, debug=False, num_devices=NCORES)

    WL_d = nc.dram_tensor("WL", [128, NJ * OD2], bf16, kind="ExternalInput").ap()
    xS0_d = nc.dram_tensor("xS0", [128, NJ * B], bf16, kind="ExternalInput").ap()
    xS1_d = nc.dram_tensor("xS1", [128, NJ * B], bf16, kind="ExternalInput").ap()
    SEL1_d = nc.dram_tensor("SEL1", [128, 32], bf16, kind="ExternalInput").ap()
    IDN_d = nc.dram_tensor("IDN", [128, 128], bf16, kind="ExternalInput").ap()
    X2_d = nc.dram_tensor("X2", [128, NJ * B], bf16, kind="ExternalInput").ap()
    vout_d = nc.dram_tensor("v_out", [B, OD2], f32, kind="ExternalOutput").ap()

    with tile.TileContext(nc) as tc:
        with (
            tc.tile_pool(name="const", bufs=1) as cp,
            tc.tile_pool(name="work", bufs=2) as wp,
            tc.tile_pool(name="small", bufs=2) as sp,
            tc.tile_pool(name="bound", bufs=1) as bp,
            tc.tile_pool(name="psum", bufs=2, space="PSUM") as pp,
            tc.tile_pool(name="pagr", bufs=2, space="PSUM") as pg,
            tc.tile_pool(name="psacc", bufs=1, space="PSUM") as pa,
            tc.tile_pool(name="dram", bufs=1, space="DRAM") as dp,
        ):
            # warm up the collective rings before the WL queue fills
            arw_in = dp.tile([128, 8], bf16, tag="arwi")
            arw_out = dp.tile([128, 8], bf16, tag="arwo")
            warmt = cp.tile([128, 8], bf16, tag="warmt")
            nc.vector.memset(warmt[:, :], 0.0)
            nc.sync.dma_start(out=arw_in[:, :], in_=warmt[:, :])
            nc.gpsimd.collective_compute(
                "AllReduce", ALU.add,
                replica_groups=[list(range(NCORES))],
                ins=[arw_in.opt()], outs=[arw_out.opt()],
            )

            # ---- resident inputs ----
            wl = cp.tile([128, NJ * OD2], bf16, tag="wl")
            for blk in range(16):
                w = NJ * OD2 // 16
                nc.sync.dma_start(out=wl[:, blk * w:(blk + 1) * w],
                                  in_=WL_d[:, blk * w:(blk + 1) * w])
            xs = [cp.tile([128, NJ * B], bf16, tag=f"xs{s}", name=f"xs{s}") for s in range(2)]
            nc.sync.dma_start(out=xs[0][:, :], in_=xS0_d[:, :])
            nc.sync.dma_start(out=xs[1][:, :], in_=xS1_d[:, :])
            sel1 = cp.tile([128, 32], bf16, tag="sel1")
            nc.sync.dma_start(out=sel1[:, :], in_=SEL1_d[:, :])
            idn = cp.tile([128, 128], bf16, tag="idn")
            nc.sync.dma_start(out=idn[:, :], in_=IDN_d[:, :])
            x2t = cp.tile([128, NJ * B], bf16, tag="x2t")
            nc.sync.dma_start(out=x2t[:, :], in_=X2_d[:, :])

            # ---- persistent state ----
            V4 = cp.tile([128, OD2], bf16, tag="V4")    # Vacc replicated x4 part-groups

            ar_in = [dp.tile([128, 512], bf16, tag=f"ari{t}", name=f"ari{t}") for t in range(NUM_ROUTES)]
            ar_out = [dp.tile([128, 512], bf16, tag=f"aro{t}", name=f"aro{t}") for t in range(NUM_ROUTES)]

            def emit_quad(t, q):
                """PE u_hat quad q + ACT evacuation -> uhsb (bf16, (d,o))."""
                jj, s_ = divmod(q, 2)
                uhp = [pp.tile([128, 1024], f32, tag="uhp", name=f"uhp{t}_{q}_{h}")
                       for h in range(2)]
                for h in range(2):
                    for ch in range(2):
                        col = jj * OD2 + (2 * h + ch) * 512
                        for r in range(4):
                            nc.tensor.matmul(
                                uhp[h][32 * r:32 * r + 32, ch * 512:(ch + 1) * 512],
                                lhsT=xs[s_][32 * r:32 * r + 32, jj * B:(jj + 1) * B],
                                rhs=wl[32 * r:32 * r + 32, col: col + 512],
                                start=True, stop=True,
                                tile_position=(32 * r, 32 * r),
                            )
                uhsb = wp.tile([128, OD2], bf16, tag="uhb", name=f"uhsb{t}_{q}",
                               bufs=4)
                for h in range(2):
                    nc.scalar.copy(uhsb[:, h * 1024:(h + 1) * 1024], uhp[h][:, :])
                return uhsb

            def emit_mul(t, q, uhsb):
                """DVE tmp = uhsb * V4 (bf16 2x)."""
                tmp = wp.tile([128, OD2], bf16, tag="tmp", name=f"tmp{t}_{q}")
                nc.vector.tensor_tensor(out=tmp[:, :], in0=uhsb[:, :], in1=V4[:, :],
                                        op=ALU.mult)
                return tmp

            def allreduce_s(t, src_psum):
                """Evacuate packed s (psum [128,512] f32) -> bf16 allreduce."""
                s_sb = cp.tile([128, 512], bf16, tag="ssb", name=f"s_sb{t}")
                nc.scalar.copy(s_sb[:, :], src_psum[:, :])
                nc.sync.dma_start(out=ar_in[t][:, :], in_=s_sb[:, :])
                nc.gpsimd.collective_compute(
                    "AllReduce", ALU.add,
                    replica_groups=[list(range(NCORES))],
                    ins=[ar_in[t].opt()],
                    outs=[ar_out[t].opt()],
                )
                # one strided DMA: dram [(ch,b), 512] -> sbuf [32, (ch,512)]
                spk = bp.tile([B, OD2], bf16, tag="spk", name=f"spk{t}")
                nc.sync.dma_start(
                    out=spk[:, :].rearrange("p (c j) -> p c j", c=4),
                    in_=ar_out[t][:, :].rearrange("(c p) j -> p c j", c=4))
                return spk

            def squash(t, s_sb):
                """v_t = squash(s_sb [32,2048] bf16, (d,o) cols).
                n2 folds on the PE (identity MMs); |s|=exp(0.5*ln(n2)) stays
                on the natural_log_exp table set (no set thrash); V4 updated
                by 4 replicate-DMAs (accum_op=add for t==1)."""
                sq = bp.tile([B, OD2], bf16, tag="sqv", name=f"sq{t}")
                nc.vector.tensor_tensor(out=sq[:, :], in0=s_sb[:, :], in1=s_sb[:, :],
                                        op=ALU.mult)
                sqv = sq[:, :].rearrange("p (d o) -> p d o", o=OC)
                n2p = pg.tile([B, OC], f32, tag="n2p", name=f"n2p{t}", bufs=1)
                for d in range(OD):
                    nc.tensor.matmul(n2p[:, 0:OC], lhsT=idn[0:B, 0:B],
                                     rhs=sqv[:, d, :],
                                     start=(d == 0), stop=(d == OD - 1),
                                     tile_position=(0, 0))
                den = bp.tile([B, OC], f32, tag="den", name=f"den_{t}")
                nc.vector.tensor_scalar_add(den[:, :], n2p[:, 0:OC], 1.0)
                rec = bp.tile([B, OC], f32, tag="rec", name=f"rec_{t}")
                nc.vector.reciprocal(rec[:, :], den[:, :])
                lnn = bp.tile([B, OC], f32, tag="lnn", name=f"ln_{t}")
                nc.scalar.activation(lnn[:, :], n2p[:, 0:OC], ACTF.Ln)
                rt = bp.tile([B, OC], f32, tag="rt", name=f"rt_{t}")
                nc.scalar.activation(rt[:, :], lnn[:, :], ACTF.Exp, scale=0.5)
                qq = bp.tile([B, OC], bf16, tag="qq", name=f"qq_{t}")
                nc.vector.tensor_tensor(out=qq[:, :], in0=rt[:, :], in1=rec[:, :],
                                        op=ALU.mult)  # |s|/(1+n2)
                qbc = qq[:, :].unsqueeze(1).broadcast_to([B, OD, OC])
                sv = s_sb[:, :].rearrange("p (d o) -> p d o", o=OC)
                if t == NUM_ROUTES - 1:
                    # write v in (o,d) order so the output DMA is contiguous
                    vt = bp.tile([B, OD2], f32, tag="vtf", name="vt_f")
                    nc.vector.tensor_tensor(
                        out=vt[:, :].rearrange("p (o d) -> p d o", d=OD),
                        in0=sv, in1=qbc, op=ALU.mult)
                    nc.sync.dma_start(out=vout_d[:, :], in_=vt[:, :])
                else:
                    vt = bp.tile([B, OD2], bf16, tag="vtb", name=f"vt{t}")
                    nc.vector.tensor_tensor(
                        out=vt[:, :].rearrange("p (d o) -> p d o", o=OC),
                        in0=sv, in1=qbc, op=ALU.mult)
                    for g in range(4):
                        if t > 0:
                            nc.gpsimd.dma_start(out=V4[32 * g:32 * g + 32, :],
                                                in_=vt[:, :], accum_op=ALU.add)
                        else:
                            nc.sync.dma_start(out=V4[32 * g:32 * g + 32, :],
                                              in_=vt[:, :])

            # ======== pass 1: s0 = sum_i u_hat / 64 (dense over (i8,k)) ========
            sacc = pa.tile([128, 512], f32, tag="sacc", name="sacc0")
            for tau in range(NJ):
                for ch in range(4):
                    nc.tensor.matmul(
                        sacc[32 * ch:32 * ch + 32, :],
                        lhsT=x2t[:, tau * B:(tau + 1) * B],
                        rhs=wl[:, tau * OD2 + ch * 512: tau * OD2 + (ch + 1) * 512],
                        start=(tau == 0), stop=(tau == NJ - 1),
                        tile_position=(0, 32 * ch))
            spk = allreduce_s(0, sacc)
            squash(0, spk)

            # ======== passes 2..3: fused agreement/softmax/s, sw-pipelined ====
            for t in range(1, NUM_ROUTES):
                sacc = pa.tile([128, 512], f32, tag="sacc", name=f"sacc{t}")
                uhsb_next = emit_quad(t, 0)          # prologue: quad 0
                tmp_next = emit_mul(t, 0, uhsb_next)
                tmp2_prev = None

                def emit_sel(q, tmp2):
                    for ch in range(4):
                        nc.tensor.matmul(
                            sacc[32 * ch:32 * ch + 32, :], lhsT=sel1[:, :],
                            rhs=tmp2[:, ch * 512:(ch + 1) * 512],
                            start=(q == 0), stop=(q == NQ - 1),
                            tile_position=(0, 32 * ch))

                for q in range(NQ):
                    uhsb, tmp = uhsb_next, tmp_next
                    if q + 1 < NQ:
                        uhsb_next = emit_quad(t, q + 1)
                    # --- PE: fold tmp over d into f32 agr (32 identity MMs) ---
                    agr = pg.tile([128, 64], f32, tag="agr", name=f"agr{t}_{q}")
                    tv = tmp[:, :].rearrange("p (d o) -> p d o", o=OC)
                    for d in range(OD):
                        nc.tensor.matmul(agr[:, 0:64], lhsT=idn[:, :], rhs=tv[:, d, :],
                                         start=(d == 0), stop=(d == OD - 1),
                                         tile_position=(0, 0))
                    # --- PE: selector MMs for q-1 (tmp2 ready -> no PE stall) ---
                    if tmp2_prev is not None:
                        emit_sel(q - 1, tmp2_prev)
                    # --- ACT: softmax numerator + Z straight off PSUM ---
                    eB = sp.tile([128, OC], bf16, tag="eB", name=f"eB{t}_{q}")
                    Zs = sp.tile([128, 1], f32, tag="Zs", name=f"Zs{t}_{q}")
                    nc.scalar.activation(eB[:, :], agr[:, 0:64], ACTF.Exp,
                                         accum_out=Zs[:, :])
                    # --- DVE: mul(q+1) fills the fold/exp gap ---
                    if q + 1 < NQ:
                        tmp_next = emit_mul(t, q + 1, uhsb_next)
                    rZ = sp.tile([128, 1], f32, tag="rZ", name=f"rZ{t}_{q}")
                    nc.vector.reciprocal(rZ[:, :], Zs[:, :])
                    epp = sp.tile([128, OC], bf16, tag="epp", name=f"epp{t}_{q}")
                    nc.vector.tensor_scalar_mul(epp[:, :], eB[:, :], rZ[:, :])
                    # --- DVE: tmp2 = uhsb * c (broadcast over outer d: 2x) ---
                    tmp2 = wp.tile([128, OD2], bf16, tag="tmp2", name=f"tmp2_{t}_{q}")
                    nc.vector.tensor_tensor(
                        out=tmp2[:, :].rearrange("p (d o) -> p d o", o=OC),
                        in0=uhsb[:, :].rearrange("p (d o) -> p d o", o=OC),
                        in1=epp[:, :].unsqueeze(1).broadcast_to([128, OD, OC]),
                        op=ALU.mult)
                    tmp2_prev = tmp2
                emit_sel(NQ - 1, tmp2_prev)
                spk = allreduce_s(t, sacc)
                squash(t, spk)

    nc.compile()
    return nc


def _host_inputs(x, W):
    """Per-core input maps (host-side relayout, not device time)."""
    W0 = np.asarray(W)[0]                       # [IC, OC, OD, KD]
    x = np.asarray(x)                           # [B, IC, KD]
    in_maps = []
    sel1 = np.zeros((128, 32), np.float32)
    for p in range(128):
        sel1[p, p % 32] = 1.0
    idn = np.eye(128, dtype=np.float32)
    for c in range(NCORES):
        # W layout: partition 16*i8 + k, col tau*2048 + d*64 + o  ((d,o) order)
        Wc = (W0[c * ICC:(c + 1) * ICC]
              .reshape(NJ, 8, OC, OD, KD))                      # [tau, i8, o, d, k]
        WL = np.ascontiguousarray(Wc.transpose(1, 4, 0, 3, 2)   # [i8, k, tau, d, o]
                                  ).reshape(128, NJ * OD2)
        xc = x[:, c * ICC:(c + 1) * ICC, :].reshape(B, NJ, 8, KD)   # [b, tau, i8, k]
        xss = []
        for s in range(2):
            Xs = np.zeros((4, 2, KD, NJ, B), np.float32)            # [r, s', k, tau, b]
            Xs[:, s] = xc[:, :, s::2].transpose(2, 3, 1, 0)         # [r, k, tau, b]
            xss.append(Xs.reshape(128, NJ * B))
        X2 = (np.ascontiguousarray(xc.transpose(2, 3, 1, 0))        # [i8, k, tau, b]
              .reshape(128, NJ * B) / float(OC))
        in_maps.append({
            "WL": WL.astype(ml_dtypes.bfloat16),
            "xS0": xss[0].astype(ml_dtypes.bfloat16),
            "xS1": xss[1].astype(ml_dtypes.bfloat16),
            "SEL1": sel1.astype(ml_dtypes.bfloat16),
            "IDN": idn.astype(ml_dtypes.bfloat16),
            "X2": X2.astype(ml_dtypes.bfloat16),
        })
    return in_maps


def kernel(x, W, _want_trace=False):
    from concourse.bass_utils import run_bass_kernel_spmd

    if "nc" not in _CACHE:
        _CACHE["nc"] = _build_program()
    nc = _CACHE["nc"]
    in_maps = _host_inputs(x, W)
    res = run_bass_kernel_spmd(nc, in_maps, core_ids=list(range(NCORES)),
                               trace=_want_trace)
    _CACHE["last_result"] = res
    out = np.asarray(res.results[0]["v_out"], np.float32)
    return out.reshape(B, OC, OD)
